# revision 26
# baseline (speedup 1.0000x reference)
"""Trainium2 Bass kernel for nn_EruSelfAttentionModel (B=4,S=1024,E=1024,A=64,H=16,L=2).

Sharding: 8 cores; core c handles batch c//2 and heads (c%2)*8..(c%2)*8+8.
Heads are fully independent through both layers, so each core runs its
(batch, 8-head) slice end-to-end with no collectives.

v2: all matmuls run with bf16 operands (fp32 is 4 cycles/row on the PE,
bf16 is 1), the inter-layer activation stays in SBUF per head (no DRAM
round-trip), softmax 1/z is folded into wT before the PV matmul, and the
final output is stored bf16 and upcast on host. PSUM accumulation stays
fp32 throughout; LayerNorm stats are computed in fp32.

Per-core dataflow (transposed [E,S] activation layout so the attention
chain needs no per-head transposes):
  stage0: indirect-DMA embedding gather -> LayerNorm (bn_stats, [S,E]
          layout, fp32) -> bf16 PE transpose -> hn0T [E,S] in SBUF
          (shared by all 8 heads' L1)
  per head: L1 attention (qkT / v / scoresT / exp / z ones-matmul /
          wT*=1/z / outT) -> ht [E,S] bf16 in SBUF -> LN (ones-matmul
          stats) -> L2 attention, final out emitted in [S,E] layout
          directly (lhsT/rhs swap) -> bf16 DRAM out.
"""

import math
import os
from contextlib import ExitStack

import numpy as np

# The device path (bass2jax under axon) needs the axon PJRT backend; a
# JAX_PLATFORMS=cpu pin (common for running the jax reference) would break it.
if "JAX_PLATFORMS" in os.environ and "axon" not in os.environ["JAX_PLATFORMS"]:
    del os.environ["JAX_PLATFORMS"]

import bass_rust
from bass_rust import SyncInfo
import concourse.bass as bass
import concourse.mybir as mybir
import concourse.tile as tile
from concourse.bass_utils import run_bass_kernel_spmd
from concourse.masks import make_identity

B, S, E, A, H, L, V = 4, 1024, 1024, 64, 16, 2, 32000
EPS = 1e-5
SCALE = math.sqrt(E)
P = 128
KO = E // P       # 8 k-blocks over E
SB = S // P       # 8 s-blocks
NH = H // 2       # 8 heads per core
HALF = S // 2     # 512
FP = mybir.dt.float32
BF = mybir.dt.bfloat16
BF_NP = mybir.dt.np(mybir.dt.bfloat16)
AF = mybir.ActivationFunctionType
OP = mybir.AluOpType

_WID = [0]


def _legalize_multi_waits(nc, max_keep=1):
    """This walrus build accepts at most one sync-wait command per engine
    instruction; split extras into standalone EventSemaphore waits."""
    for f in nc.m.functions:
        for blk in f.blocks:
            out = []
            changed = False
            for inst in blk.instructions:
                si = inst.sync_info
                ow = list(si.on_wait) if si is not None else []
                if len(ow) > max_keep:
                    changed = True
                    for w in ow[:-max_keep]:
                        _WID[0] += 1
                        out.append(bass_rust.InstEventSemaphore(
                            name=f"WSPLIT-{_WID[0]}",
                            engine=inst.engine,
                            ins=[], outs=[],
                            sync_info=SyncInfo(on_wait=[w], on_update=[]),
                        ))
                    inst.sync_info = SyncInfo(on_wait=ow[-max_keep:],
                                              on_update=list(si.on_update))
                out.append(inst)
            if changed:
                blk.instructions = out


def _build_nc(g0_identity, legalize=True):
    nc = bass.Bass("TRN2")

    emb = nc.dram_tensor("emb", [V, E], FP, kind="ExternalInput")
    xidx = nc.dram_tensor("xidx", [S, 1], mybir.dt.int32, kind="ExternalInput")
    wqk = nc.dram_tensor("wqk", [L, NH, E, 2 * A], BF, kind="ExternalInput")
    wv = nc.dram_tensor("wv", [L, NH, E, E], BF, kind="ExternalInput")
    g0 = nc.dram_tensor("g0", [E], FP, kind="ExternalInput")
    b0 = nc.dram_tensor("b0", [E], FP, kind="ExternalInput")
    out_d = nc.dram_tensor("out", [NH, S, E], BF, kind="ExternalOutput")
    rstd_d = nc.dram_tensor("rstd_scratch", [NH, S], FP)  # Internal bounce

    with tile.TileContext(nc) as tc, ExitStack() as ctx:
        const = ctx.enter_context(tc.tile_pool(name="const", bufs=1))
        hnp = ctx.enter_context(tc.tile_pool(name="hnp", bufs=1))
        h0p = ctx.enter_context(tc.tile_pool(name="h0p", bufs=2))
        h0np = ctx.enter_context(tc.tile_pool(name="h0np", bufs=2))
        wvp = ctx.enter_context(tc.tile_pool(name="wvp", bufs=2))
        wqkp = ctx.enter_context(tc.tile_pool(name="wqkp", bufs=2))
        vp = ctx.enter_context(tc.tile_pool(name="vp", bufs=2))
        wtp = ctx.enter_context(tc.tile_pool(name="wtp", bufs=2))
        qkp = ctx.enter_context(tc.tile_pool(name="qkp", bufs=2))
        htp = ctx.enter_context(tc.tile_pool(name="htp", bufs=2))
        outp = ctx.enter_context(tc.tile_pool(name="outp", bufs=3))
        statp = ctx.enter_context(tc.tile_pool(name="statp", bufs=1))
        smallp = ctx.enter_context(tc.tile_pool(name="smallp", bufs=2))
        psA = ctx.enter_context(tc.tile_pool(name="psA", bufs=2, space="PSUM"))
        psS = ctx.enter_context(tc.tile_pool(name="psS", bufs=2, space="PSUM"))
        psZO = ctx.enter_context(tc.tile_pool(name="psZO", bufs=2, space="PSUM"))

        identB = const.tile([P, P], BF)
        make_identity(nc, identB[:])
        identF = const.tile([P, P], FP)
        make_identity(nc, identF[:])
        ones128 = const.tile([P, P], BF)
        nc.vector.memset(ones128[:], 1.0)
        oneEb = const.tile([P, P], BF)
        nc.vector.memset(oneEb[:], 1.0 / E)
        eps_t = const.tile([P, 1], FP)
        nc.vector.memset(eps_t[:], EPS)
        if not g0_identity:
            g0rep = const.tile([P, E], FP)
            b0rep = const.tile([P, E], FP)
            nc.sync.dma_start(g0rep[:], g0.ap()[None, :].to_broadcast([P, E]))
            nc.sync.dma_start(b0rep[:], b0.ap()[None, :].to_broadcast([P, E]))

        # ---------------- stage 0: embed + LN0 + transpose -> hn0T ----------
        hn0T = hnp.tile([P, KO, S], BF, tag="hn")  # [e_inner, e_outer, s]
        for sb in range(SB):
            h0sb = h0p.tile([P, E], FP, tag="h0sb")
            idxt = smallp.tile([P, 1], mybir.dt.int32, tag="idx")
            nc.sync.dma_start(idxt[:], xidx[sb * P:(sb + 1) * P, :])
            nc.gpsimd.indirect_dma_start(
                out=h0sb[:], out_offset=None, in_=emb[:, :],
                in_offset=bass.IndirectOffsetOnAxis(ap=idxt[:, :1], axis=0),
            )
            # LayerNorm over free dim (E) via bn_stats, fp32
            stats = smallp.tile([P, 2, 6], FP, tag="bnst")
            nc.vector.bn_stats(stats[:, 0, :], h0sb[:, 0:512])
            nc.vector.bn_stats(stats[:, 1, :], h0sb[:, 512:1024])
            mv = smallp.tile([P, 2], FP, tag="bnmv")
            nc.vector.bn_aggr(mv[:], stats[:])
            rstd = smallp.tile([P, 1], FP, tag="rstd")
            nc.scalar.activation(rstd[:], mv[:, 1:2], AF.Sqrt, bias=eps_t[:])
            nc.vector.reciprocal(rstd[:], rstd[:])
            h0n = h0np.tile([P, E], BF, tag="h0n")
            if g0_identity:
                nc.vector.tensor_scalar(h0n[:], h0sb[:], scalar1=mv[:, 0:1],
                                        scalar2=rstd[:], op0=OP.subtract,
                                        op1=OP.mult)
            else:
                nc.vector.tensor_scalar(h0sb[:], h0sb[:], scalar1=mv[:, 0:1],
                                        scalar2=rstd[:], op0=OP.subtract,
                                        op1=OP.mult)
                nc.vector.tensor_tensor(h0sb[:], h0sb[:], g0rep[:], OP.mult)
                nc.vector.tensor_tensor(h0n[:], h0sb[:], b0rep[:], OP.add)
            for eo in range(KO):
                pst = psS.tile([P, P], BF, tag="ps_s")
                nc.tensor.transpose(pst[:], h0n[:, eo * P:(eo + 1) * P],
                                    identB[:])
                nc.any.tensor_copy(hn0T[:, eo, sb * P:(sb + 1) * P], pst[:])

        # ---------------- attention unit ------------------------------------
        def attn_unit(layer, head, hn, ht_dst, final, rstdf=None, rstd_col=None):
            """One attention layer for one head. With host-side column-centered
            weights, LayerNorm reduces to the rstd factor: when rstdf/rstd_col
            are given, hn is the RAW (uncentered, unscaled) activation and
            rstd is folded into the q/k and v PSUM drains."""
            # weight DMAs ride the Activation HWDGE queue so they never queue
            # behind the output DMAs on the SP queue
            wqk_sb = wqkp.tile([P, KO, 2 * A], BF, tag="wqk")
            nc.scalar.dma_start(wqk_sb[:],
                                wqk.ap()[layer, head].rearrange("(ko p) m -> p ko m", p=P))
            wv_sb = wvp.tile([P, KO, E], BF, tag="wv")
            nc.scalar.dma_start(wv_sb[:],
                                wv.ap()[layer, head].rearrange("(ko p) o -> p ko o", p=P))

            # qkT: [2A=128, S] packed q (rows 0:64) and k (rows 64:128)
            ps_qk = psA.tile([P, S], FP, tag="big")
            for nb in range(2):
                for ko in range(KO):
                    nc.tensor.matmul(ps_qk[:, nb * 512:(nb + 1) * 512],
                                     lhsT=wqk_sb[:, ko, :],
                                     rhs=hn[:, ko, nb * 512:(nb + 1) * 512],
                                     start=(ko == 0), stop=(ko == KO - 1))
            qT = qkp.tile([A, S], BF, tag="qT")
            kT = qkp.tile([A, S], BF, tag="kT")
            if rstdf is None:
                nc.any.tensor_copy(qT[:], ps_qk[0:A, :])
                nc.any.tensor_copy(kT[:], ps_qk[A:2 * A, :])
            else:
                nc.vector.tensor_tensor(qT[:], ps_qk[0:A, :], rstdf[0:A, :],
                                        OP.mult)
                nc.vector.tensor_tensor(kT[:], ps_qk[A:2 * A, :],
                                        rstdf[A:2 * A, :], OP.mult)

            # v: [T, O] (t on partitions), bf16
            v_sb = vp.tile([P, SB, E], BF, tag="v")
            for tb in range(SB):
                ps_v = psA.tile([P, E], FP, tag="big")
                for nb in range(2):
                    for ko in range(KO):
                        nc.tensor.matmul(ps_v[:, nb * 512:(nb + 1) * 512],
                                         lhsT=hn[:, ko, tb * P:(tb + 1) * P],
                                         rhs=wv_sb[:, ko, nb * 512:(nb + 1) * 512],
                                         start=(ko == 0), stop=(ko == KO - 1))
                if rstd_col is None:
                    nc.any.tensor_copy(v_sb[:, tb, :], ps_v[:])
                else:
                    nc.vector.tensor_scalar_mul(v_sb[:, tb, :], ps_v[:],
                                                rstd_col[:, tb:tb + 1])

            for sh in range(2):
                s0 = sh * HALF
                # scoresT + exp -> wT [T, s-half], bf16. The partial tile-sum
                # for z is interleaved with the exp pipeline so it completes
                # right after the last exp; 1/z is applied on the PSUM drain
                # of the PV matmul so the out matmuls never wait on it.
                wT = wtp.tile([P, SB, HALF], BF, tag="wt")
                zsum = smallp.tile([P, HALF], BF, tag="zsum")
                for tb in range(SB):
                    ps_s = psS.tile([P, HALF], FP, tag="ps_s")
                    nc.tensor.matmul(ps_s[:], lhsT=kT[:, tb * P:(tb + 1) * P],
                                     rhs=qT[:, s0:s0 + HALF], start=True, stop=True)
                    nc.scalar.activation(wT[:, tb, :], ps_s[:], AF.Exp,
                                         scale=float(1.0 / SCALE))
                    if tb == 1:
                        nc.vector.tensor_tensor(zsum[:], wT[:, 0, :], wT[:, 1, :],
                                                OP.add)
                    elif tb > 1:
                        nc.vector.tensor_tensor(zsum[:], zsum[:], wT[:, tb, :],
                                                OP.add)
                ps_z = psS.tile([P, HALF], FP, tag="ps_s")
                nc.tensor.matmul(ps_z[:], lhsT=ones128[:], rhs=zsum[:],
                                 start=True, stop=True)
                invzf = smallp.tile([P, HALF], FP, tag="invzf")
                nc.vector.reciprocal(invzf[:], ps_z[:])

                if not final:
                    # outT [O, s-half] -> ht (SBUF, bf16, [E,S] layout)
                    for ob in range(KO):
                        ps_o = psZO.tile([P, HALF], FP, tag="zo")
                        for tb in range(SB):
                            nc.tensor.matmul(ps_o[:],
                                             lhsT=v_sb[:, tb, ob * P:(ob + 1) * P],
                                             rhs=wT[:, tb, :],
                                             start=(tb == 0), stop=(tb == SB - 1))
                        nc.vector.tensor_tensor(ht_dst[:, ob, s0:s0 + HALF],
                                                ps_o[:], invzf[:], OP.mult)
                else:
                    # per-partition 1/z column via PE transpose of invzf blocks
                    izc = smallp.tile([P, 4], FP, tag="izc")
                    for sbb in range(4):
                        pst = psS.tile([P, P], FP, tag="ps_s")
                        nc.tensor.transpose(pst[:], invzf[:, sbb * P:(sbb + 1) * P],
                                            identF[:])
                        nc.any.tensor_copy(izc[:, sbb:sbb + 1], pst[:, 0:1])
                    # out [s, O] directly (lhsT = wT slice) -> bf16 DRAM,
                    # one [P, E] DMA per s-block
                    for sbb in range(4):
                        ot = outp.tile([P, E], BF, tag="ot")
                        for nb in range(2):
                            ps_o = psZO.tile([P, HALF], FP, tag="zo")
                            for tb in range(SB):
                                nc.tensor.matmul(ps_o[:],
                                                 lhsT=wT[:, tb, sbb * P:(sbb + 1) * P],
                                                 rhs=v_sb[:, tb, nb * 512:(nb + 1) * 512],
                                                 start=(tb == 0), stop=(tb == SB - 1))
                            nc.vector.tensor_scalar_mul(ot[:, nb * 512:(nb + 1) * 512],
                                                        ps_o[:], izc[:, sbb:sbb + 1])
                        nc.sync.dma_start(
                            out_d.ap()[head, (sh * 4 + sbb) * P:(sh * 4 + sbb + 1) * P, :],
                            ot[:])

        # ---------------- layer 2 rstd ([E,S] layout, stats via matmul) -----
        # With column-centered weights the mean-subtraction is implicit in the
        # L2 matmuls; only 1/std is needed. Returned replicated across
        # partitions ([P,S] fp32) and as per-t-block columns ([P,SB] fp32).
        def l2_sums(ht):
            """Free-dim partial sums for the L2 stats: pure DVE/Scalar work
            (no tensor-queue instructions), issued BEFORE the next head's L1
            so it runs concurrently with those matmuls."""
            husum = statp.tile([P, S], BF, tag="husum")
            nc.vector.tensor_tensor(husum[:], ht[:, 0, :], ht[:, 1, :], OP.add)
            for ko in range(2, KO):
                nc.vector.tensor_tensor(husum[:], husum[:], ht[:, ko, :], OP.add)
            sqsum = statp.tile([P, S], BF, tag="sqsum")
            nc.scalar.square(sqsum[:], ht[:, 0, :])
            for ko in range(1, KO):
                sqt = smallp.tile([P, S], BF, tag="sqt")
                nc.scalar.square(sqt[:], ht[:, ko, :])
                nc.vector.tensor_tensor(sqsum[:], sqsum[:], sqt[:], OP.add)
            return husum, sqsum

        def l2_rstd(husum, sqsum, head):
            rstdf = statp.tile([P, S], FP, tag="rstdf")
            for nb in range(2):
                sl = slice(nb * 512, (nb + 1) * 512)
                mu_ps = psS.tile([P, HALF], FP, tag="ps_s")
                nc.tensor.matmul(mu_ps[:], lhsT=oneEb[:], rhs=husum[:, sl],
                                 start=True, stop=True)
                sq_ps = psS.tile([P, HALF], FP, tag="ps_s")
                nc.tensor.matmul(sq_ps[:], lhsT=oneEb[:], rhs=sqsum[:, sl],
                                 start=True, stop=True)
                varf = smallp.tile([P, HALF], FP, tag="varf")
                nc.scalar.square(varf[:], mu_ps[:])
                nc.vector.tensor_tensor(varf[:], sq_ps[:], varf[:], OP.subtract)
                nc.scalar.activation(varf[:], varf[:], AF.Sqrt, bias=eps_t[:])
                nc.vector.reciprocal(rstdf[:, sl], varf[:])
            # per-partition column form for the v drain, via a DRAM bounce
            # (repartitioning DMA) so it stays off the compute queues.
            rstd_col = statp.tile([P, SB], FP, tag="rstd_col")
            nc.scalar.dma_start(rstd_d.ap()[head], rstdf[0:1, :])
            nc.scalar.dma_start(rstd_col[:, :],
                                rstd_d.ap()[head].rearrange("(t p) -> p t", p=P))
            return rstdf, rstd_col

        # ---------------- per-head fused L1 -> rstd -> L2 ---------------------
        # Software-pipelined: L1 of head h+1 is issued before stats(h)+L2(h)
        # so the next head's matmuls keep the in-order tensor queue busy while
        # the serial stats chain runs on the vector/scalar engines.
        ht_cur = htp.tile([P, KO, S], BF, tag="ht")
        attn_unit(0, 0, hn0T, ht_cur, final=False)
        for head in range(NH):
            husum, sqsum = l2_sums(ht_cur)
            if head + 1 < NH:
                ht_next = htp.tile([P, KO, S], BF, tag="ht")
                attn_unit(0, head + 1, hn0T, ht_next, final=False)
            else:
                ht_next = None
            rstdf, rstd_col = l2_rstd(husum, sqsum, head)
            attn_unit(1, head, ht_cur, None, final=True,
                      rstdf=rstdf, rstd_col=rstd_col)
            ht_cur = ht_next

    if legalize:
        _legalize_multi_waits(nc)
    return nc


_CACHE = {}


def _get_nc(g0_identity, legalize=True):
    key = (g0_identity, legalize)
    if key not in _CACHE:
        _CACHE[key] = _build_nc(g0_identity, legalize)
    return _CACHE[key]


def _prep_in_maps(x, emb, ln_gamma, ln_beta, Wq, Wk, Wv):
    x = np.asarray(x)
    emb = np.ascontiguousarray(np.asarray(emb, dtype=np.float32))
    ln_gamma = np.asarray(ln_gamma, dtype=np.float32)
    ln_beta = np.asarray(ln_beta, dtype=np.float32)
    Wq = np.asarray(Wq, dtype=np.float32).copy()
    Wk = np.asarray(Wk, dtype=np.float32).copy()
    Wv = np.asarray(Wv, dtype=np.float32).copy()

    g0_id = bool(np.all(ln_gamma[0] == 1.0) and np.all(ln_beta[0] == 0.0))
    g1_id = bool(np.all(ln_gamma[1] == 1.0) and np.all(ln_beta[1] == 0.0))

    # Fold LayerNorm mean-subtraction into the weights: q/k/v contract the
    # normalized activation over e, and sum_e n_e = 0, so centering each
    # weight column over e is exactly the -mu term. Layer 2 always reads the
    # RAW activation on-device (centering required); layer 1 reads the
    # on-chip-normalized hn0 (sum_e hn0 = 0 only when LN0 is identity, so
    # centering is a harmless accuracy improvement there).
    if not g1_id:
        if not np.all(ln_beta[1] == 0.0):
            raise NotImplementedError("nonzero LN2 beta is not supported")
        Wq[1] *= ln_gamma[1][None, None, :]
        Wk[1] *= ln_gamma[1][None, None, :]
        Wv[1] *= ln_gamma[1][None, None, :]
    lcent = [1] + ([0] if g0_id else [])
    for W in (Wq, Wk, Wv):
        for l in lcent:
            W[l] -= W[l].mean(axis=-1, keepdims=True)

    # [L,H,E,2A] packed (WqT | WkT); [L,H,E,E] = WvT; bf16 for the PE
    wqkT = np.concatenate([Wq.transpose(0, 1, 3, 2), Wk.transpose(0, 1, 3, 2)],
                          axis=3).astype(BF_NP)
    wvT = Wv.transpose(0, 1, 3, 2).astype(BF_NP)

    in_maps = []
    for c in range(8):
        b = c // 2
        hs = (c % 2) * NH
        in_maps.append({
            "emb": emb,
            "xidx": np.ascontiguousarray(x[b].astype(np.int32).reshape(S, 1)),
            "wqk": np.ascontiguousarray(wqkT[:, hs:hs + NH]),
            "wv": np.ascontiguousarray(wvT[:, hs:hs + NH]),
            "g0": np.ascontiguousarray(ln_gamma[0]),
            "b0": np.ascontiguousarray(ln_beta[0]),
        })
    return in_maps, g0_id


def run(inputs, trace=False, trace_cores=None):
    in_maps, g0_id = _prep_in_maps(**inputs)
    nc = _get_nc(g0_id)
    res = run_bass_kernel_spmd(nc, in_maps, core_ids=list(range(8)),
                               trace=trace, trace_cores=trace_cores)
    out = np.empty((B, H, S, E), dtype=np.float32)
    for c in range(8):
        out[c // 2, (c % 2) * NH:(c % 2) * NH + NH] = \
            res.results[c]["out"].astype(np.float32)
    return out, res


def kernel(x, emb, ln_gamma, ln_beta, Wq, Wk, Wv):
    out, _ = run(dict(x=x, emb=emb, ln_gamma=ln_gamma, ln_beta=ln_beta,
                      Wq=Wq, Wk=Wk, Wv=Wv))
    return out


# revision 27
# speedup vs baseline: 1.2447x; 1.2447x over previous
"""Trainium2 Bass kernel for nn_EruSelfAttentionModel (B=4,S=1024,E=1024,A=64,H=16,L=2).

Sharding: 8 cores; core c handles batch c//2 and heads (c%2)*8..(c%2)*8+8.
Heads are fully independent through both layers, so each core runs its
(batch, 8-head) slice end-to-end with no collectives.

v2: all matmuls run with bf16 operands (fp32 is 4 cycles/row on the PE,
bf16 is 1), the inter-layer activation stays in SBUF per head (no DRAM
round-trip), softmax 1/z is folded into wT before the PV matmul, and the
final output is stored bf16 and upcast on host. PSUM accumulation stays
fp32 throughout; LayerNorm stats are computed in fp32.

Per-core dataflow (transposed [E,S] activation layout so the attention
chain needs no per-head transposes):
  stage0: indirect-DMA embedding gather -> LayerNorm (bn_stats, [S,E]
          layout, fp32) -> bf16 PE transpose -> hn0T [E,S] in SBUF
          (shared by all 8 heads' L1)
  per head: L1 attention (qkT / v / scoresT / exp / z ones-matmul /
          wT*=1/z / outT) -> ht [E,S] bf16 in SBUF -> LN (ones-matmul
          stats) -> L2 attention, final out emitted in [S,E] layout
          directly (lhsT/rhs swap) -> bf16 DRAM out.
"""

import math
import os
from contextlib import ExitStack

import numpy as np

# The device path (bass2jax under axon) needs the axon PJRT backend; a
# JAX_PLATFORMS=cpu pin (common for running the jax reference) would break it.
if "JAX_PLATFORMS" in os.environ and "axon" not in os.environ["JAX_PLATFORMS"]:
    del os.environ["JAX_PLATFORMS"]

import bass_rust
from bass_rust import SyncInfo
import concourse.bass as bass
import concourse.mybir as mybir
import concourse.tile as tile
from concourse.bass_utils import run_bass_kernel_spmd
from concourse.masks import make_identity

B, S, E, A, H, L, V = 4, 1024, 1024, 64, 16, 2, 32000
EPS = 1e-5
SCALE = math.sqrt(E)
P = 128
KO = E // P       # 8 k-blocks over E
SB = S // P       # 8 s-blocks
NH = H // 2       # 8 heads per core
HALF = S // 2     # 512
FP = mybir.dt.float32
BF = mybir.dt.bfloat16
BF_NP = mybir.dt.np(mybir.dt.bfloat16)
AF = mybir.ActivationFunctionType
OP = mybir.AluOpType

_WID = [0]


def _legalize_multi_waits(nc, max_keep=1):
    """This walrus build accepts at most one sync-wait command per engine
    instruction; split extras into standalone EventSemaphore waits."""
    for f in nc.m.functions:
        for blk in f.blocks:
            out = []
            changed = False
            for inst in blk.instructions:
                si = inst.sync_info
                ow = list(si.on_wait) if si is not None else []
                if len(ow) > max_keep:
                    changed = True
                    for w in ow[:-max_keep]:
                        _WID[0] += 1
                        out.append(bass_rust.InstEventSemaphore(
                            name=f"WSPLIT-{_WID[0]}",
                            engine=inst.engine,
                            ins=[], outs=[],
                            sync_info=SyncInfo(on_wait=[w], on_update=[]),
                        ))
                    inst.sync_info = SyncInfo(on_wait=ow[-max_keep:],
                                              on_update=list(si.on_update))
                out.append(inst)
            if changed:
                blk.instructions = out


def _build_nc(g0_identity, legalize=True):
    nc = bass.Bass("TRN2")

    emb = nc.dram_tensor("emb", [V, E], FP, kind="ExternalInput")
    xidx = nc.dram_tensor("xidx", [S, 1], mybir.dt.int32, kind="ExternalInput")
    wqk = nc.dram_tensor("wqk", [L, NH, E, 2 * A], BF, kind="ExternalInput")
    wv = nc.dram_tensor("wv", [L, NH, E, E], BF, kind="ExternalInput")
    g0 = nc.dram_tensor("g0", [E], FP, kind="ExternalInput")
    b0 = nc.dram_tensor("b0", [E], FP, kind="ExternalInput")
    out_d = nc.dram_tensor("out", [NH, S, E], BF, kind="ExternalOutput")
    rstd_d = nc.dram_tensor("rstd_scratch", [NH, S], FP)  # Internal bounce

    with tile.TileContext(nc) as tc, ExitStack() as ctx:
        const = ctx.enter_context(tc.tile_pool(name="const", bufs=1))
        hnp = ctx.enter_context(tc.tile_pool(name="hnp", bufs=1))
        h0p = ctx.enter_context(tc.tile_pool(name="h0p", bufs=2))
        h0np = ctx.enter_context(tc.tile_pool(name="h0np", bufs=2))
        wvp = ctx.enter_context(tc.tile_pool(name="wvp", bufs=2))
        wqkp = ctx.enter_context(tc.tile_pool(name="wqkp", bufs=2))
        vp = ctx.enter_context(tc.tile_pool(name="vp", bufs=2))
        wtp = ctx.enter_context(tc.tile_pool(name="wtp", bufs=2))
        qkp = ctx.enter_context(tc.tile_pool(name="qkp", bufs=2))
        htp = ctx.enter_context(tc.tile_pool(name="htp", bufs=2))
        outp = ctx.enter_context(tc.tile_pool(name="outp", bufs=3))
        statp = ctx.enter_context(tc.tile_pool(name="statp", bufs=1))
        smallp = ctx.enter_context(tc.tile_pool(name="smallp", bufs=2))
        psA = ctx.enter_context(tc.tile_pool(name="psA", bufs=2, space="PSUM"))
        psS = ctx.enter_context(tc.tile_pool(name="psS", bufs=2, space="PSUM"))
        psZO = ctx.enter_context(tc.tile_pool(name="psZO", bufs=2, space="PSUM"))

        identB = const.tile([P, P], BF)
        make_identity(nc, identB[:])
        identF = const.tile([P, P], FP)
        make_identity(nc, identF[:])
        ones128 = const.tile([P, P], BF)
        nc.vector.memset(ones128[:], 1.0)
        oneEb = const.tile([P, P], BF)
        nc.vector.memset(oneEb[:], 1.0 / E)
        eps_t = const.tile([P, 1], FP)
        nc.vector.memset(eps_t[:], EPS)
        if not g0_identity:
            g0rep = const.tile([P, E], FP)
            b0rep = const.tile([P, E], FP)
            nc.sync.dma_start(g0rep[:], g0.ap()[None, :].to_broadcast([P, E]))
            nc.sync.dma_start(b0rep[:], b0.ap()[None, :].to_broadcast([P, E]))

        # ---------------- stage 0: embed + LN0 + transpose -> hn0T ----------
        hn0T = hnp.tile([P, KO, S], BF, tag="hn")  # [e_inner, e_outer, s]
        for sb in range(SB):
            h0sb = h0p.tile([P, E], FP, tag="h0sb")
            idxt = smallp.tile([P, 1], mybir.dt.int32, tag="idx")
            nc.sync.dma_start(idxt[:], xidx[sb * P:(sb + 1) * P, :])
            nc.gpsimd.indirect_dma_start(
                out=h0sb[:], out_offset=None, in_=emb[:, :],
                in_offset=bass.IndirectOffsetOnAxis(ap=idxt[:, :1], axis=0),
            )
            # LayerNorm over free dim (E) via bn_stats, fp32
            stats = smallp.tile([P, 2, 6], FP, tag="bnst")
            nc.vector.bn_stats(stats[:, 0, :], h0sb[:, 0:512])
            nc.vector.bn_stats(stats[:, 1, :], h0sb[:, 512:1024])
            mv = smallp.tile([P, 2], FP, tag="bnmv")
            nc.vector.bn_aggr(mv[:], stats[:])
            rstd = smallp.tile([P, 1], FP, tag="rstd")
            nc.scalar.activation(rstd[:], mv[:, 1:2], AF.Sqrt, bias=eps_t[:])
            nc.vector.reciprocal(rstd[:], rstd[:])
            h0n = h0np.tile([P, E], BF, tag="h0n")
            if g0_identity:
                nc.vector.tensor_scalar(h0n[:], h0sb[:], scalar1=mv[:, 0:1],
                                        scalar2=rstd[:], op0=OP.subtract,
                                        op1=OP.mult)
            else:
                nc.vector.tensor_scalar(h0sb[:], h0sb[:], scalar1=mv[:, 0:1],
                                        scalar2=rstd[:], op0=OP.subtract,
                                        op1=OP.mult)
                nc.vector.tensor_tensor(h0sb[:], h0sb[:], g0rep[:], OP.mult)
                nc.vector.tensor_tensor(h0n[:], h0sb[:], b0rep[:], OP.add)
            for eo in range(KO):
                pst = psS.tile([P, P], BF, tag="ps_s")
                nc.tensor.transpose(pst[:], h0n[:, eo * P:(eo + 1) * P],
                                    identB[:])
                nc.any.tensor_copy(hn0T[:, eo, sb * P:(sb + 1) * P], pst[:])

        # ---------------- attention unit ------------------------------------
        def attn_unit(layer, head, hn, ht_dst, final, rstdf=None, rstd_col=None):
            """One attention layer for one head. With host-side column-centered
            weights, LayerNorm reduces to the rstd factor: when rstdf/rstd_col
            are given, hn is the RAW (uncentered, unscaled) activation and
            rstd is folded into the q/k and v PSUM drains."""
            wqk_sb = wqkp.tile([P, KO, 2 * A], BF, tag="wqk")
            nc.sync.dma_start(wqk_sb[:],
                              wqk.ap()[layer, head].rearrange("(ko p) m -> p ko m", p=P))
            wv_sb = wvp.tile([P, KO, E], BF, tag="wv")
            nc.sync.dma_start(wv_sb[:],
                              wv.ap()[layer, head].rearrange("(ko p) o -> p ko o", p=P))

            # qkT: [2A=128, S] packed q (rows 0:64) and k (rows 64:128)
            ps_qk = psA.tile([P, S], FP, tag="big")
            for nb in range(2):
                for ko in range(KO):
                    nc.tensor.matmul(ps_qk[:, nb * 512:(nb + 1) * 512],
                                     lhsT=wqk_sb[:, ko, :],
                                     rhs=hn[:, ko, nb * 512:(nb + 1) * 512],
                                     start=(ko == 0), stop=(ko == KO - 1))
            qT = qkp.tile([A, S], BF, tag="qT")
            kT = qkp.tile([A, S], BF, tag="kT")
            if rstdf is None:
                nc.any.tensor_copy(qT[:], ps_qk[0:A, :])
                nc.any.tensor_copy(kT[:], ps_qk[A:2 * A, :])
            else:
                nc.vector.tensor_tensor(qT[:], ps_qk[0:A, :], rstdf[0:A, :],
                                        OP.mult)
                nc.vector.tensor_tensor(kT[:], ps_qk[A:2 * A, :],
                                        rstdf[A:2 * A, :], OP.mult)

            # v: [T, O] (t on partitions), bf16
            v_sb = vp.tile([P, SB, E], BF, tag="v")
            for tb in range(SB):
                ps_v = psA.tile([P, E], FP, tag="big")
                for nb in range(2):
                    for ko in range(KO):
                        nc.tensor.matmul(ps_v[:, nb * 512:(nb + 1) * 512],
                                         lhsT=hn[:, ko, tb * P:(tb + 1) * P],
                                         rhs=wv_sb[:, ko, nb * 512:(nb + 1) * 512],
                                         start=(ko == 0), stop=(ko == KO - 1))
                if rstd_col is None:
                    nc.any.tensor_copy(v_sb[:, tb, :], ps_v[:])
                else:
                    nc.vector.tensor_scalar_mul(v_sb[:, tb, :], ps_v[:],
                                                rstd_col[:, tb:tb + 1])

            for sh in range(2):
                s0 = sh * HALF
                # scoresT + exp -> wT [T, s-half], bf16. The partial tile-sum
                # for z is interleaved with the exp pipeline so it completes
                # right after the last exp; 1/z is applied on the PSUM drain
                # of the PV matmul so the out matmuls never wait on it.
                wT = wtp.tile([P, SB, HALF], BF, tag="wt")
                zsum = smallp.tile([P, HALF], BF, tag="zsum")
                for tb in range(SB):
                    ps_s = psS.tile([P, HALF], FP, tag="ps_s")
                    nc.tensor.matmul(ps_s[:], lhsT=kT[:, tb * P:(tb + 1) * P],
                                     rhs=qT[:, s0:s0 + HALF], start=True, stop=True)
                    nc.scalar.activation(wT[:, tb, :], ps_s[:], AF.Exp,
                                         scale=float(1.0 / SCALE))
                    if tb == 1:
                        nc.vector.tensor_tensor(zsum[:], wT[:, 0, :], wT[:, 1, :],
                                                OP.add)
                    elif tb > 1:
                        nc.vector.tensor_tensor(zsum[:], zsum[:], wT[:, tb, :],
                                                OP.add)
                ps_z = psS.tile([P, HALF], FP, tag="ps_s")
                nc.tensor.matmul(ps_z[:], lhsT=ones128[:], rhs=zsum[:],
                                 start=True, stop=True)
                invzf = smallp.tile([P, HALF], FP, tag="invzf")
                nc.vector.reciprocal(invzf[:], ps_z[:])

                if not final:
                    # outT [O, s-half] -> ht (SBUF, bf16, [E,S] layout)
                    for ob in range(KO):
                        ps_o = psZO.tile([P, HALF], FP, tag="zo")
                        for tb in range(SB):
                            nc.tensor.matmul(ps_o[:],
                                             lhsT=v_sb[:, tb, ob * P:(ob + 1) * P],
                                             rhs=wT[:, tb, :],
                                             start=(tb == 0), stop=(tb == SB - 1))
                        nc.vector.tensor_tensor(ht_dst[:, ob, s0:s0 + HALF],
                                                ps_o[:], invzf[:], OP.mult)
                else:
                    # per-partition 1/z column via PE transpose of invzf blocks
                    izc = smallp.tile([P, 4], FP, tag="izc")
                    for sbb in range(4):
                        pst = psS.tile([P, P], FP, tag="ps_s")
                        nc.tensor.transpose(pst[:], invzf[:, sbb * P:(sbb + 1) * P],
                                            identF[:])
                        nc.any.tensor_copy(izc[:, sbb:sbb + 1], pst[:, 0:1])
                    # out [s, O] directly (lhsT = wT slice) -> bf16 DRAM,
                    # one [P, E] DMA per s-block
                    for sbb in range(4):
                        ot = outp.tile([P, E], BF, tag="ot")
                        for nb in range(2):
                            ps_o = psZO.tile([P, HALF], FP, tag="zo")
                            for tb in range(SB):
                                nc.tensor.matmul(ps_o[:],
                                                 lhsT=wT[:, tb, sbb * P:(sbb + 1) * P],
                                                 rhs=v_sb[:, tb, nb * 512:(nb + 1) * 512],
                                                 start=(tb == 0), stop=(tb == SB - 1))
                            nc.vector.tensor_scalar_mul(ot[:, nb * 512:(nb + 1) * 512],
                                                        ps_o[:], izc[:, sbb:sbb + 1])
                        nc.sync.dma_start(
                            out_d.ap()[head, (sh * 4 + sbb) * P:(sh * 4 + sbb + 1) * P, :],
                            ot[:])

        # ---------------- layer 2 rstd ([E,S] layout, stats via matmul) -----
        # With column-centered weights the mean-subtraction is implicit in the
        # L2 matmuls; only 1/std is needed. Returned replicated across
        # partitions ([P,S] fp32) and as per-t-block columns ([P,SB] fp32).
        def l2_sums(ht):
            """Free-dim partial sums for the L2 stats: pure DVE/Scalar work
            (no tensor-queue instructions), issued BEFORE the next head's L1
            so it runs concurrently with those matmuls."""
            husum = statp.tile([P, S], BF, tag="husum")
            nc.vector.tensor_tensor(husum[:], ht[:, 0, :], ht[:, 1, :], OP.add)
            for ko in range(2, KO):
                nc.vector.tensor_tensor(husum[:], husum[:], ht[:, ko, :], OP.add)
            sqsum = statp.tile([P, S], BF, tag="sqsum")
            nc.scalar.square(sqsum[:], ht[:, 0, :])
            for ko in range(1, KO):
                sqt = smallp.tile([P, S], BF, tag="sqt")
                nc.scalar.square(sqt[:], ht[:, ko, :])
                nc.vector.tensor_tensor(sqsum[:], sqsum[:], sqt[:], OP.add)
            return husum, sqsum

        def l2_rstd(husum, sqsum, head):
            rstdf = statp.tile([P, S], FP, tag="rstdf")
            for nb in range(2):
                sl = slice(nb * 512, (nb + 1) * 512)
                mu_ps = psS.tile([P, HALF], FP, tag="ps_s")
                nc.tensor.matmul(mu_ps[:], lhsT=oneEb[:], rhs=husum[:, sl],
                                 start=True, stop=True)
                sq_ps = psS.tile([P, HALF], FP, tag="ps_s")
                nc.tensor.matmul(sq_ps[:], lhsT=oneEb[:], rhs=sqsum[:, sl],
                                 start=True, stop=True)
                varf = smallp.tile([P, HALF], FP, tag="varf")
                nc.scalar.square(varf[:], mu_ps[:])
                nc.vector.tensor_tensor(varf[:], sq_ps[:], varf[:], OP.subtract)
                nc.scalar.activation(varf[:], varf[:], AF.Sqrt, bias=eps_t[:])
                nc.vector.reciprocal(rstdf[:, sl], varf[:])
            # per-partition column form for the v drain, via a DRAM bounce
            # (repartitioning DMA) so it stays off the compute queues.
            rstd_col = statp.tile([P, SB], FP, tag="rstd_col")
            nc.sync.dma_start(rstd_d.ap()[head], rstdf[0:1, :])
            nc.sync.dma_start(rstd_col[:, :],
                              rstd_d.ap()[head].rearrange("(t p) -> p t", p=P))
            return rstdf, rstd_col

        # ---------------- per-head fused L1 -> rstd -> L2 ---------------------
        # Software-pipelined: L1 of head h+1 is issued before stats(h)+L2(h)
        # so the next head's matmuls keep the in-order tensor queue busy while
        # the serial stats chain runs on the vector/scalar engines.
        ht_cur = htp.tile([P, KO, S], BF, tag="ht")
        attn_unit(0, 0, hn0T, ht_cur, final=False)
        for head in range(NH):
            husum, sqsum = l2_sums(ht_cur)
            if head + 1 < NH:
                ht_next = htp.tile([P, KO, S], BF, tag="ht")
                attn_unit(0, head + 1, hn0T, ht_next, final=False)
            else:
                ht_next = None
            rstdf, rstd_col = l2_rstd(husum, sqsum, head)
            attn_unit(1, head, ht_cur, None, final=True,
                      rstdf=rstdf, rstd_col=rstd_col)
            ht_cur = ht_next

    if legalize:
        _legalize_multi_waits(nc)
    return nc


_CACHE = {}


def _get_nc(g0_identity, legalize=True):
    key = (g0_identity, legalize)
    if key not in _CACHE:
        _CACHE[key] = _build_nc(g0_identity, legalize)
    return _CACHE[key]


def _prep_in_maps(x, emb, ln_gamma, ln_beta, Wq, Wk, Wv):
    x = np.asarray(x)
    emb = np.ascontiguousarray(np.asarray(emb, dtype=np.float32))
    ln_gamma = np.asarray(ln_gamma, dtype=np.float32)
    ln_beta = np.asarray(ln_beta, dtype=np.float32)
    Wq = np.asarray(Wq, dtype=np.float32).copy()
    Wk = np.asarray(Wk, dtype=np.float32).copy()
    Wv = np.asarray(Wv, dtype=np.float32).copy()

    g0_id = bool(np.all(ln_gamma[0] == 1.0) and np.all(ln_beta[0] == 0.0))
    g1_id = bool(np.all(ln_gamma[1] == 1.0) and np.all(ln_beta[1] == 0.0))

    # Fold LayerNorm mean-subtraction into the weights: q/k/v contract the
    # normalized activation over e, and sum_e n_e = 0, so centering each
    # weight column over e is exactly the -mu term. Layer 2 always reads the
    # RAW activation on-device (centering required); layer 1 reads the
    # on-chip-normalized hn0 (sum_e hn0 = 0 only when LN0 is identity, so
    # centering is a harmless accuracy improvement there).
    if not g1_id:
        if not np.all(ln_beta[1] == 0.0):
            raise NotImplementedError("nonzero LN2 beta is not supported")
        Wq[1] *= ln_gamma[1][None, None, :]
        Wk[1] *= ln_gamma[1][None, None, :]
        Wv[1] *= ln_gamma[1][None, None, :]
    lcent = [1] + ([0] if g0_id else [])
    for W in (Wq, Wk, Wv):
        for l in lcent:
            W[l] -= W[l].mean(axis=-1, keepdims=True)

    # [L,H,E,2A] packed (WqT | WkT); [L,H,E,E] = WvT; bf16 for the PE
    wqkT = np.concatenate([Wq.transpose(0, 1, 3, 2), Wk.transpose(0, 1, 3, 2)],
                          axis=3).astype(BF_NP)
    wvT = Wv.transpose(0, 1, 3, 2).astype(BF_NP)

    in_maps = []
    for c in range(8):
        b = c // 2
        hs = (c % 2) * NH
        in_maps.append({
            "emb": emb,
            "xidx": np.ascontiguousarray(x[b].astype(np.int32).reshape(S, 1)),
            "wqk": np.ascontiguousarray(wqkT[:, hs:hs + NH]),
            "wv": np.ascontiguousarray(wvT[:, hs:hs + NH]),
            "g0": np.ascontiguousarray(ln_gamma[0]),
            "b0": np.ascontiguousarray(ln_beta[0]),
        })
    return in_maps, g0_id


def run(inputs, trace=False, trace_cores=None):
    in_maps, g0_id = _prep_in_maps(**inputs)
    nc = _get_nc(g0_id)
    res = run_bass_kernel_spmd(nc, in_maps, core_ids=list(range(8)),
                               trace=trace, trace_cores=trace_cores)
    out = np.empty((B, H, S, E), dtype=np.float32)
    for c in range(8):
        out[c // 2, (c % 2) * NH:(c % 2) * NH + NH] = \
            res.results[c]["out"].astype(np.float32)
    return out, res


def kernel(x, emb, ln_gamma, ln_beta, Wq, Wk, Wv):
    out, _ = run(dict(x=x, emb=emb, ln_gamma=ln_gamma, ln_beta=ln_beta,
                      Wq=Wq, Wk=Wk, Wv=Wv))
    return out


# revision 30
# speedup vs baseline: 1.2508x; 1.0049x over previous
"""Trainium2 Bass kernel for nn_EruSelfAttentionModel (B=4,S=1024,E=1024,A=64,H=16,L=2).

Sharding: 8 cores; core c handles batch c//2 and heads (c%2)*8..(c%2)*8+8.
Heads are fully independent through both layers, so each core runs its
(batch, 8-head) slice end-to-end with no collectives.

v2: all matmuls run with bf16 operands (fp32 is 4 cycles/row on the PE,
bf16 is 1), the inter-layer activation stays in SBUF per head (no DRAM
round-trip), softmax 1/z is folded into wT before the PV matmul, and the
final output is stored bf16 and upcast on host. PSUM accumulation stays
fp32 throughout; LayerNorm stats are computed in fp32.

Per-core dataflow (transposed [E,S] activation layout so the attention
chain needs no per-head transposes):
  stage0: indirect-DMA embedding gather -> LayerNorm (bn_stats, [S,E]
          layout, fp32) -> bf16 PE transpose -> hn0T [E,S] in SBUF
          (shared by all 8 heads' L1)
  per head: L1 attention (qkT / v / scoresT / exp / z ones-matmul /
          wT*=1/z / outT) -> ht [E,S] bf16 in SBUF -> LN (ones-matmul
          stats) -> L2 attention, final out emitted in [S,E] layout
          directly (lhsT/rhs swap) -> bf16 DRAM out.
"""

import math
import os
from contextlib import ExitStack

import numpy as np

# The device path (bass2jax under axon) needs the axon PJRT backend; a
# JAX_PLATFORMS=cpu pin (common for running the jax reference) would break it.
if "JAX_PLATFORMS" in os.environ and "axon" not in os.environ["JAX_PLATFORMS"]:
    del os.environ["JAX_PLATFORMS"]

import bass_rust
from bass_rust import SyncInfo
import concourse.bass as bass
import concourse.mybir as mybir
import concourse.tile as tile
from concourse.bass_utils import run_bass_kernel_spmd
from concourse.masks import make_identity

B, S, E, A, H, L, V = 4, 1024, 1024, 64, 16, 2, 32000
EPS = 1e-5
SCALE = math.sqrt(E)
P = 128
KO = E // P       # 8 k-blocks over E
SB = S // P       # 8 s-blocks
NH = H // 2       # 8 heads per core
HALF = S // 2     # 512
FP = mybir.dt.float32
BF = mybir.dt.bfloat16
BF_NP = mybir.dt.np(mybir.dt.bfloat16)
AF = mybir.ActivationFunctionType
OP = mybir.AluOpType

_WID = [0]


def _legalize_multi_waits(nc, max_keep=1):
    """This walrus build accepts at most one sync-wait command per engine
    instruction; split extras into standalone EventSemaphore waits."""
    for f in nc.m.functions:
        for blk in f.blocks:
            out = []
            changed = False
            for inst in blk.instructions:
                si = inst.sync_info
                ow = list(si.on_wait) if si is not None else []
                if len(ow) > max_keep:
                    changed = True
                    for w in ow[:-max_keep]:
                        _WID[0] += 1
                        out.append(bass_rust.InstEventSemaphore(
                            name=f"WSPLIT-{_WID[0]}",
                            engine=inst.engine,
                            ins=[], outs=[],
                            sync_info=SyncInfo(on_wait=[w], on_update=[]),
                        ))
                    inst.sync_info = SyncInfo(on_wait=ow[-max_keep:],
                                              on_update=list(si.on_update))
                out.append(inst)
            if changed:
                blk.instructions = out


def _build_nc(g0_identity, legalize=True):
    nc = bass.Bass("TRN2")

    emb = nc.dram_tensor("emb", [V, E], FP, kind="ExternalInput")
    xidx = nc.dram_tensor("xidx", [S, 1], mybir.dt.int32, kind="ExternalInput")
    wqk = nc.dram_tensor("wqk", [L, NH, E, 2 * A], BF, kind="ExternalInput")
    wv = nc.dram_tensor("wv", [L, NH, E, E], BF, kind="ExternalInput")
    g0 = nc.dram_tensor("g0", [E], FP, kind="ExternalInput")
    b0 = nc.dram_tensor("b0", [E], FP, kind="ExternalInput")
    out_d = nc.dram_tensor("out", [NH, S, E], BF, kind="ExternalOutput")
    rstd_d = nc.dram_tensor("rstd_scratch", [NH, S], FP)  # Internal bounce

    with tile.TileContext(nc) as tc, ExitStack() as ctx:
        const = ctx.enter_context(tc.tile_pool(name="const", bufs=1))
        hnp = ctx.enter_context(tc.tile_pool(name="hnp", bufs=1))
        h0p = ctx.enter_context(tc.tile_pool(name="h0p", bufs=4))
        h0np = ctx.enter_context(tc.tile_pool(name="h0np", bufs=2))
        wvp = ctx.enter_context(tc.tile_pool(name="wvp", bufs=2))
        wqkp = ctx.enter_context(tc.tile_pool(name="wqkp", bufs=2))
        vp = ctx.enter_context(tc.tile_pool(name="vp", bufs=2))
        wtp = ctx.enter_context(tc.tile_pool(name="wtp", bufs=2))
        qkp = ctx.enter_context(tc.tile_pool(name="qkp", bufs=2))
        htp = ctx.enter_context(tc.tile_pool(name="htp", bufs=2))
        outp = ctx.enter_context(tc.tile_pool(name="outp", bufs=3))
        statp = ctx.enter_context(tc.tile_pool(name="statp", bufs=1))
        smallp = ctx.enter_context(tc.tile_pool(name="smallp", bufs=2))
        psA = ctx.enter_context(tc.tile_pool(name="psA", bufs=2, space="PSUM"))
        psS = ctx.enter_context(tc.tile_pool(name="psS", bufs=2, space="PSUM"))
        psZO = ctx.enter_context(tc.tile_pool(name="psZO", bufs=2, space="PSUM"))

        identB = const.tile([P, P], BF)
        make_identity(nc, identB[:])
        identF = const.tile([P, P], FP)
        make_identity(nc, identF[:])
        ones128 = const.tile([P, P], BF)
        nc.vector.memset(ones128[:], 1.0)
        oneEb = const.tile([P, P], BF)
        nc.vector.memset(oneEb[:], 1.0 / E)
        eps_t = const.tile([P, 1], FP)
        nc.vector.memset(eps_t[:], EPS)
        if not g0_identity:
            g0rep = const.tile([P, E], FP)
            b0rep = const.tile([P, E], FP)
            nc.sync.dma_start(g0rep[:], g0.ap()[None, :].to_broadcast([P, E]))
            nc.sync.dma_start(b0rep[:], b0.ap()[None, :].to_broadcast([P, E]))

        # ---------------- stage 0: embed + LN0 + transpose -> hn0T ----------
        hn0T = hnp.tile([P, KO, S], BF, tag="hn")  # [e_inner, e_outer, s]
        for sb in range(SB):
            h0sb = h0p.tile([P, E], FP, tag="h0sb")
            idxt = smallp.tile([P, 1], mybir.dt.int32, tag="idx")
            nc.sync.dma_start(idxt[:], xidx[sb * P:(sb + 1) * P, :])
            nc.gpsimd.indirect_dma_start(
                out=h0sb[:], out_offset=None, in_=emb[:, :],
                in_offset=bass.IndirectOffsetOnAxis(ap=idxt[:, :1], axis=0),
            )
            # LayerNorm over free dim (E) via bn_stats, fp32
            stats = smallp.tile([P, 2, 6], FP, tag="bnst")
            nc.vector.bn_stats(stats[:, 0, :], h0sb[:, 0:512])
            nc.vector.bn_stats(stats[:, 1, :], h0sb[:, 512:1024])
            mv = smallp.tile([P, 2], FP, tag="bnmv")
            nc.vector.bn_aggr(mv[:], stats[:])
            rstd = smallp.tile([P, 1], FP, tag="rstd")
            nc.scalar.activation(rstd[:], mv[:, 1:2], AF.Sqrt, bias=eps_t[:])
            nc.vector.reciprocal(rstd[:], rstd[:])
            h0n = h0np.tile([P, E], BF, tag="h0n")
            if g0_identity:
                nc.vector.tensor_scalar(h0n[:], h0sb[:], scalar1=mv[:, 0:1],
                                        scalar2=rstd[:], op0=OP.subtract,
                                        op1=OP.mult)
            else:
                nc.vector.tensor_scalar(h0sb[:], h0sb[:], scalar1=mv[:, 0:1],
                                        scalar2=rstd[:], op0=OP.subtract,
                                        op1=OP.mult)
                nc.vector.tensor_tensor(h0sb[:], h0sb[:], g0rep[:], OP.mult)
                nc.vector.tensor_tensor(h0n[:], h0sb[:], b0rep[:], OP.add)
            for eo in range(KO):
                pst = psS.tile([P, P], BF, tag="ps_s")
                nc.tensor.transpose(pst[:], h0n[:, eo * P:(eo + 1) * P],
                                    identB[:])
                nc.any.tensor_copy(hn0T[:, eo, sb * P:(sb + 1) * P], pst[:])

        # ---------------- attention unit ------------------------------------
        def attn_unit(layer, head, hn, ht_dst, final, rstdf=None, rstd_col=None):
            """One attention layer for one head. With host-side column-centered
            weights, LayerNorm reduces to the rstd factor: when rstdf/rstd_col
            are given, hn is the RAW (uncentered, unscaled) activation and
            rstd is folded into the q/k and v PSUM drains."""
            wqk_sb = wqkp.tile([P, KO, 2 * A], BF, tag="wqk")
            nc.sync.dma_start(wqk_sb[:],
                              wqk.ap()[layer, head].rearrange("(ko p) m -> p ko m", p=P))
            wv_sb = wvp.tile([P, KO, E], BF, tag="wv")
            nc.sync.dma_start(wv_sb[:],
                              wv.ap()[layer, head].rearrange("(ko p) o -> p ko o", p=P))

            # qkT: [2A=128, S] packed q (rows 0:64) and k (rows 64:128)
            ps_qk = psA.tile([P, S], FP, tag="big")
            for nb in range(2):
                for ko in range(KO):
                    nc.tensor.matmul(ps_qk[:, nb * 512:(nb + 1) * 512],
                                     lhsT=wqk_sb[:, ko, :],
                                     rhs=hn[:, ko, nb * 512:(nb + 1) * 512],
                                     start=(ko == 0), stop=(ko == KO - 1))
            qT = qkp.tile([A, S], BF, tag="qT")
            kT = qkp.tile([A, S], BF, tag="kT")
            if rstdf is None:
                nc.any.tensor_copy(qT[:], ps_qk[0:A, :])
                nc.any.tensor_copy(kT[:], ps_qk[A:2 * A, :])
            else:
                nc.vector.tensor_tensor(qT[:], ps_qk[0:A, :], rstdf[0:A, :],
                                        OP.mult)
                nc.vector.tensor_tensor(kT[:], ps_qk[A:2 * A, :],
                                        rstdf[A:2 * A, :], OP.mult)

            # v: [T, O] (t on partitions), bf16
            v_sb = vp.tile([P, SB, E], BF, tag="v")
            for tb in range(SB):
                ps_v = psA.tile([P, E], FP, tag="big")
                for nb in range(2):
                    for ko in range(KO):
                        nc.tensor.matmul(ps_v[:, nb * 512:(nb + 1) * 512],
                                         lhsT=hn[:, ko, tb * P:(tb + 1) * P],
                                         rhs=wv_sb[:, ko, nb * 512:(nb + 1) * 512],
                                         start=(ko == 0), stop=(ko == KO - 1))
                if rstd_col is None:
                    nc.any.tensor_copy(v_sb[:, tb, :], ps_v[:])
                else:
                    nc.vector.tensor_scalar_mul(v_sb[:, tb, :], ps_v[:],
                                                rstd_col[:, tb:tb + 1])

            for sh in range(2):
                s0 = sh * HALF
                # scoresT + exp -> wT [T, s-half], bf16. The partial tile-sum
                # for z is interleaved with the exp pipeline so it completes
                # right after the last exp; 1/z is applied on the PSUM drain
                # of the PV matmul so the out matmuls never wait on it.
                wT = wtp.tile([P, SB, HALF], BF, tag="wt")
                zsum = smallp.tile([P, HALF], BF, tag="zsum")
                for tb in range(SB):
                    ps_s = psS.tile([P, HALF], FP, tag="ps_s")
                    nc.tensor.matmul(ps_s[:], lhsT=kT[:, tb * P:(tb + 1) * P],
                                     rhs=qT[:, s0:s0 + HALF], start=True, stop=True)
                    nc.scalar.activation(wT[:, tb, :], ps_s[:], AF.Exp,
                                         scale=float(1.0 / SCALE))
                    if tb == 1:
                        nc.vector.tensor_tensor(zsum[:], wT[:, 0, :], wT[:, 1, :],
                                                OP.add)
                    elif tb > 1:
                        nc.vector.tensor_tensor(zsum[:], zsum[:], wT[:, tb, :],
                                                OP.add)
                # The z ones-matmul is issued AFTER the first PV accumulation
                # group so the tensor queue never waits on the exp pipeline;
                # the reciprocal runs while the later PV groups stream.
                invzf = smallp.tile([P, HALF], FP, tag="invzf")

                def z_chain():
                    ps_z = psS.tile([P, HALF], FP, tag="ps_s")
                    nc.tensor.matmul(ps_z[:], lhsT=ones128[:], rhs=zsum[:],
                                     start=True, stop=True)
                    nc.vector.reciprocal(invzf[:], ps_z[:])

                if not final:
                    # outT [O, s-half] -> ht (SBUF, bf16, [E,S] layout)
                    for ob in range(KO):
                        ps_o = psZO.tile([P, HALF], FP, tag="zo")
                        for tb in range(SB):
                            nc.tensor.matmul(ps_o[:],
                                             lhsT=v_sb[:, tb, ob * P:(ob + 1) * P],
                                             rhs=wT[:, tb, :],
                                             start=(tb == 0), stop=(tb == SB - 1))
                        if ob == 0:
                            z_chain()
                        nc.vector.tensor_tensor(ht_dst[:, ob, s0:s0 + HALF],
                                                ps_o[:], invzf[:], OP.mult)
                else:
                    # out [s, O] directly (lhsT = wT slice) -> bf16 DRAM,
                    # one [P, E] DMA per s-block; per-partition 1/z column via
                    # PE transpose of invzf blocks, issued after the second
                    # PV group so the transposes don't stall the PE.
                    izc = smallp.tile([P, 4], FP, tag="izc")
                    for sbb in range(4):
                        ot = outp.tile([P, E], BF, tag="ot")
                        ps_held = []
                        for nb in range(2):
                            ps_o = psZO.tile([P, HALF], FP, tag="zo")
                            for tb in range(SB):
                                nc.tensor.matmul(ps_o[:],
                                                 lhsT=wT[:, tb, sbb * P:(sbb + 1) * P],
                                                 rhs=v_sb[:, tb, nb * 512:(nb + 1) * 512],
                                                 start=(tb == 0), stop=(tb == SB - 1))
                            if sbb == 0 and nb == 0:
                                z_chain()
                            if sbb == 0:
                                ps_held.append(ps_o)  # drain after izc exists
                            else:
                                nc.vector.tensor_scalar_mul(
                                    ot[:, nb * 512:(nb + 1) * 512],
                                    ps_o[:], izc[:, sbb:sbb + 1])
                        if sbb == 0:
                            for izb in range(4):
                                pst = psS.tile([P, P], FP, tag="ps_s")
                                nc.tensor.transpose(
                                    pst[:], invzf[:, izb * P:(izb + 1) * P],
                                    identF[:])
                                nc.any.tensor_copy(izc[:, izb:izb + 1],
                                                   pst[:, 0:1])
                            for nb in range(2):
                                nc.vector.tensor_scalar_mul(
                                    ot[:, nb * 512:(nb + 1) * 512],
                                    ps_held[nb][:], izc[:, 0:1])
                        nc.sync.dma_start(
                            out_d.ap()[head, (sh * 4 + sbb) * P:(sh * 4 + sbb + 1) * P, :],
                            ot[:])

        # ---------------- layer 2 rstd ([E,S] layout, stats via matmul) -----
        # With column-centered weights the mean-subtraction is implicit in the
        # L2 matmuls; only 1/std is needed. Returned replicated across
        # partitions ([P,S] fp32) and as per-t-block columns ([P,SB] fp32).
        def l2_sums(ht):
            """Free-dim partial sums for the L2 stats: pure DVE/Scalar work
            (no tensor-queue instructions), issued BEFORE the next head's L1
            so it runs concurrently with those matmuls."""
            husum = statp.tile([P, S], BF, tag="husum")
            nc.vector.tensor_tensor(husum[:], ht[:, 0, :], ht[:, 1, :], OP.add)
            for ko in range(2, KO):
                nc.vector.tensor_tensor(husum[:], husum[:], ht[:, ko, :], OP.add)
            sqsum = statp.tile([P, S], BF, tag="sqsum")
            nc.scalar.square(sqsum[:], ht[:, 0, :])
            for ko in range(1, KO):
                sqt = smallp.tile([P, S], BF, tag="sqt")
                nc.scalar.square(sqt[:], ht[:, ko, :])
                nc.vector.tensor_tensor(sqsum[:], sqsum[:], sqt[:], OP.add)
            return husum, sqsum

        def l2_rstd(husum, sqsum, head):
            rstdf = statp.tile([P, S], FP, tag="rstdf")
            for nb in range(2):
                sl = slice(nb * 512, (nb + 1) * 512)
                mu_ps = psS.tile([P, HALF], FP, tag="ps_s")
                nc.tensor.matmul(mu_ps[:], lhsT=oneEb[:], rhs=husum[:, sl],
                                 start=True, stop=True)
                sq_ps = psS.tile([P, HALF], FP, tag="ps_s")
                nc.tensor.matmul(sq_ps[:], lhsT=oneEb[:], rhs=sqsum[:, sl],
                                 start=True, stop=True)
                varf = smallp.tile([P, HALF], FP, tag="varf")
                nc.scalar.square(varf[:], mu_ps[:])
                nc.vector.tensor_tensor(varf[:], sq_ps[:], varf[:], OP.subtract)
                nc.scalar.activation(varf[:], varf[:], AF.Sqrt, bias=eps_t[:])
                nc.vector.reciprocal(rstdf[:, sl], varf[:])
            # per-partition column form for the v drain, via a DRAM bounce
            # (repartitioning DMA) so it stays off the compute queues.
            rstd_col = statp.tile([P, SB], FP, tag="rstd_col")
            nc.sync.dma_start(rstd_d.ap()[head], rstdf[0:1, :])
            nc.sync.dma_start(rstd_col[:, :],
                              rstd_d.ap()[head].rearrange("(t p) -> p t", p=P))
            return rstdf, rstd_col

        # ---------------- per-head fused L1 -> rstd -> L2 ---------------------
        # Software-pipelined: L1 of head h+1 is issued before stats(h)+L2(h)
        # so the next head's matmuls keep the in-order tensor queue busy while
        # the serial stats chain runs on the vector/scalar engines.
        ht_cur = htp.tile([P, KO, S], BF, tag="ht")
        attn_unit(0, 0, hn0T, ht_cur, final=False)
        for head in range(NH):
            husum, sqsum = l2_sums(ht_cur)
            if head + 1 < NH:
                ht_next = htp.tile([P, KO, S], BF, tag="ht")
                attn_unit(0, head + 1, hn0T, ht_next, final=False)
            else:
                ht_next = None
            rstdf, rstd_col = l2_rstd(husum, sqsum, head)
            attn_unit(1, head, ht_cur, None, final=True,
                      rstdf=rstdf, rstd_col=rstd_col)
            ht_cur = ht_next

    if legalize:
        _legalize_multi_waits(nc)
    return nc


_CACHE = {}


def _get_nc(g0_identity, legalize=True):
    key = (g0_identity, legalize)
    if key not in _CACHE:
        _CACHE[key] = _build_nc(g0_identity, legalize)
    return _CACHE[key]


def _prep_in_maps(x, emb, ln_gamma, ln_beta, Wq, Wk, Wv):
    x = np.asarray(x)
    emb = np.ascontiguousarray(np.asarray(emb, dtype=np.float32))
    ln_gamma = np.asarray(ln_gamma, dtype=np.float32)
    ln_beta = np.asarray(ln_beta, dtype=np.float32)
    Wq = np.asarray(Wq, dtype=np.float32).copy()
    Wk = np.asarray(Wk, dtype=np.float32).copy()
    Wv = np.asarray(Wv, dtype=np.float32).copy()

    g0_id = bool(np.all(ln_gamma[0] == 1.0) and np.all(ln_beta[0] == 0.0))
    g1_id = bool(np.all(ln_gamma[1] == 1.0) and np.all(ln_beta[1] == 0.0))

    # Fold LayerNorm mean-subtraction into the weights: q/k/v contract the
    # normalized activation over e, and sum_e n_e = 0, so centering each
    # weight column over e is exactly the -mu term. Layer 2 always reads the
    # RAW activation on-device (centering required); layer 1 reads the
    # on-chip-normalized hn0 (sum_e hn0 = 0 only when LN0 is identity, so
    # centering is a harmless accuracy improvement there).
    if not g1_id:
        if not np.all(ln_beta[1] == 0.0):
            raise NotImplementedError("nonzero LN2 beta is not supported")
        Wq[1] *= ln_gamma[1][None, None, :]
        Wk[1] *= ln_gamma[1][None, None, :]
        Wv[1] *= ln_gamma[1][None, None, :]
    lcent = [1] + ([0] if g0_id else [])
    for W in (Wq, Wk, Wv):
        for l in lcent:
            W[l] -= W[l].mean(axis=-1, keepdims=True)

    # [L,H,E,2A] packed (WqT | WkT); [L,H,E,E] = WvT; bf16 for the PE
    wqkT = np.concatenate([Wq.transpose(0, 1, 3, 2), Wk.transpose(0, 1, 3, 2)],
                          axis=3).astype(BF_NP)
    wvT = Wv.transpose(0, 1, 3, 2).astype(BF_NP)

    in_maps = []
    for c in range(8):
        b = c // 2
        hs = (c % 2) * NH
        in_maps.append({
            "emb": emb,
            "xidx": np.ascontiguousarray(x[b].astype(np.int32).reshape(S, 1)),
            "wqk": np.ascontiguousarray(wqkT[:, hs:hs + NH]),
            "wv": np.ascontiguousarray(wvT[:, hs:hs + NH]),
            "g0": np.ascontiguousarray(ln_gamma[0]),
            "b0": np.ascontiguousarray(ln_beta[0]),
        })
    return in_maps, g0_id


def run(inputs, trace=False, trace_cores=None):
    in_maps, g0_id = _prep_in_maps(**inputs)
    nc = _get_nc(g0_id)
    res = run_bass_kernel_spmd(nc, in_maps, core_ids=list(range(8)),
                               trace=trace, trace_cores=trace_cores)
    out = np.empty((B, H, S, E), dtype=np.float32)
    for c in range(8):
        out[c // 2, (c % 2) * NH:(c % 2) * NH + NH] = \
            res.results[c]["out"].astype(np.float32)
    return out, res


def kernel(x, emb, ln_gamma, ln_beta, Wq, Wk, Wv):
    out, _ = run(dict(x=x, emb=emb, ln_gamma=ln_gamma, ln_beta=ln_beta,
                      Wq=Wq, Wk=Wk, Wv=Wv))
    return out
